# revision 16
# baseline (speedup 1.0000x reference)
"""Sliding-window GQA attention (Gemma-style) on 8 TRN2 NeuronCores.

Sharding: tensor-parallel over heads. Core c owns q-heads {2c, 2c+1} and
kv-head c. Each core computes Q/K/V projections (+RoPE) for its heads over
the full sequence, banded sliding-window attention, then an AllToAll
(split into 2 token-chunks, overlapped with attention/output compute)
reshards the attention output by token so every core computes the full
output projection for its 512-token slice. Host concatenates slices.

All matmuls run in bf16 (f32 PSUM accumulation); softmax runs in f32.
"""

import os
import sys

for _p in ("/opt/trn_rl_repo",):
    if _p not in sys.path:
        sys.path.insert(0, _p)

import numpy as np
import ml_dtypes

import concourse.bass as bass
import concourse.mybir as mybir
import concourse.tile as tile
from concourse import bacc
from concourse.bass_utils import run_bass_kernel_spmd
from concourse.masks import make_identity

F32 = mybir.dt.float32
BF16 = mybir.dt.bfloat16
AF = mybir.ActivationFunctionType
ALU = mybir.AluOpType

B, T, D = 2, 2048, 3584
NQ, NKV, H = 16, 8, 256
SCALAR = 0.0625
SOFT_CAP = 50.0
WINDOW = 1024
ROPE_BASE = 10000.0

NCORES = 8
P = 128
DC = D // P              # 28 contraction chunks
TQ = T // P              # 16 query tiles per batch
TPIECE = 256             # projection output tile width
NPIECE = T // TPIECE
HLOC = 2 * H             # 512 local q-head columns per core
TOK = B * T              # 4096
TPC = TOK // NCORES      # 512 tokens per core after AllToAll
CHTOK = TPC // 2         # 256 tokens per A2A chunk block
WTILES = WINDOW // P     # 8
MASKVAL = -1.0e30        # added to tanh output; exp(50*(t+MASKVAL)) == 0
SEGMAX = 8               # max QK tiles per PSUM strip segment (2 banks)
NHC = NQ * H // P        # 32 global h chunks
DP = 512                 # output projection d piece

last_result = None       # BassKernelResults of the most recent device run


def _band(i, mode):
    lo = max(0, i - WTILES)
    hi = i if mode == "tril" else min(TQ - 1, i + WTILES)
    return lo, hi


def _segments(lo, hi):
    segs = []
    j = lo
    while j <= hi:
        j1 = min(j + SEGMAX - 1, hi)
        segs.append((j, j1))
        j = j1 + 1
    return segs


def build(mode):
    assert mode in ("tril", "ones")
    nseg_max = 3 if mode == "ones" else 2
    nc = bacc.Bacc("TRN2", target_bir_lowering=False, debug=False,
                   num_devices=NCORES)

    xT = nc.dram_tensor("xT", [D, TOK], BF16, kind="ExternalInput")
    wq = nc.dram_tensor("wq", [D, HLOC], BF16, kind="ExternalInput")
    wk = nc.dram_tensor("wk", [D, H], BF16, kind="ExternalInput")
    wv = nc.dram_tensor("wv", [D, H], BF16, kind="ExternalInput")
    wo = nc.dram_tensor("wo", [NHC, P, D], BF16, kind="ExternalInput")
    ropeq = nc.dram_tensor("ropeq", [2, P, T], BF16, kind="ExternalInput")
    ropek = nc.dram_tensor("ropek", [2, P, T], BF16, kind="ExternalInput")
    out = nc.dram_tensor("out", [TPC, D], F32, kind="ExternalOutput")

    with tile.TileContext(nc) as tc:
        with (
            tc.tile_pool(name="dram", bufs=1, space="DRAM") as dram,
            tc.tile_pool(name="consts", bufs=1) as consts,
            tc.tile_pool(name="qkv", bufs=1) as qkvpool,
        ):
            # A2A bounce buffers, token-major: [src_rank_block][tok][local h]
            a2a_in = [dram.tile([NCORES, CHTOK, HLOC], BF16,
                                name=f"a2a_in{m}") for m in range(2)]
            a2a_out = [dram.tile([NCORES, CHTOK, HLOC], BF16,
                                 name=f"a2a_out{m}") for m in range(2)]

            # ---- constants ----
            ident = consts.tile([P, P], F32)
            make_identity(nc, ident)
            # causal: valid (0) where k <= q, MASKVAL above diag
            causal = consts.tile([P, P], F32)
            nc.gpsimd.memset(causal, 0.0)
            nc.gpsimd.affine_select(
                out=causal, in_=causal, compare_op=ALU.is_ge, fill=MASKVAL,
                base=0, pattern=[[-1, P]], channel_multiplier=1)
            # upperstrict: valid (0) where q < k (window lower edge, j=i-8)
            upperstrict = consts.tile([P, P], F32)
            nc.gpsimd.memset(upperstrict, MASKVAL)
            nc.gpsimd.affine_select(
                out=upperstrict, in_=upperstrict, compare_op=ALU.is_ge,
                fill=0.0, base=0, pattern=[[-1, P]], channel_multiplier=1)
            # lowerstrict: valid (0) where k < q (window upper edge, j=i+8)
            lowerstrict = None
            if mode == "ones":
                lowerstrict = consts.tile([P, P], F32)
                nc.gpsimd.memset(lowerstrict, 0.0)
                nc.gpsimd.affine_select(
                    out=lowerstrict, in_=lowerstrict, compare_op=ALU.is_gt,
                    fill=MASKVAL, base=0, pattern=[[-1, P]],
                    channel_multiplier=1)

            qts, kts, vsbs = [], [], []
            xT_v = xT[:].rearrange("(c p) t -> p c t", p=P)

            # =================== projections + rope =======================
            with (
                tc.tile_pool(name="weights", bufs=1) as wpool,
                tc.tile_pool(name="xt", bufs=2) as xpool,
                tc.tile_pool(name="rtmp", bufs=2) as rpool,
                tc.tile_pool(name="proj_ps", bufs=4, space="PSUM") as ppsum,
                tc.tile_pool(name="projv_ps", bufs=2, space="PSUM") as vpsum,
            ):
                wq_sb = wpool.tile([P, DC, HLOC], BF16)
                wq_v = wq[:].rearrange("(c p) h -> p c h", p=P)
                nc.sync.dma_start(wq_sb[:, :DC // 2, :], wq_v[:, :DC // 2, :])
                nc.sync.dma_start(wq_sb[:, DC // 2:, :], wq_v[:, DC // 2:, :])
                wk_sb = wpool.tile([P, DC, H], BF16)
                nc.sync.dma_start(wk_sb[:],
                                  wk[:].rearrange("(c p) h -> p c h", p=P))
                wv_sb = wpool.tile([P, DC, H], BF16)
                nc.sync.dma_start(wv_sb[:],
                                  wv[:].rearrange("(c p) h -> p c h", p=P))
                rq_sb = wpool.tile([P, 2, T], BF16)
                nc.sync.dma_start(rq_sb[:], ropeq[:].rearrange("s p t -> p s t"))
                rk_sb = wpool.tile([P, 2, T], BF16)
                nc.sync.dma_start(rk_sb[:], ropek[:].rearrange("s p t -> p s t"))

                for b in range(B):
                    qt = qkvpool.tile([P, 4, T], BF16, tag=f"qt{b}")
                    kt = qkvpool.tile([P, 2, T], BF16, tag=f"kt{b}")
                    vsb = qkvpool.tile([P, TQ, H], BF16, tag=f"v{b}")
                    qts.append(qt)
                    kts.append(kt)
                    vsbs.append(vsb)

                    for pi in range(NPIECE):
                        t0 = pi * TPIECE
                        xt = xpool.tile([P, DC, TPIECE], BF16, tag="xt")
                        nc.sync.dma_start(
                            xt[:], xT_v[:, :, b * T + t0:b * T + t0 + TPIECE])

                        def rope(dst, hc0, psA, psB, tab, t0=t0):
                            cos = tab[:, 0, t0:t0 + TPIECE]
                            sin = tab[:, 1, t0:t0 + TPIECE]
                            t1 = rpool.tile([P, TPIECE], F32, tag="r1")
                            t2 = rpool.tile([P, TPIECE], F32, tag="r2")
                            nc.vector.tensor_tensor(t1[:], psA[:], cos, ALU.mult)
                            nc.vector.tensor_tensor(t2[:], psB[:], sin, ALU.mult)
                            nc.vector.tensor_tensor(
                                dst[:, hc0, t0:t0 + TPIECE], t1[:], t2[:],
                                ALU.subtract)
                            t3 = rpool.tile([P, TPIECE], F32, tag="r3")
                            t4 = rpool.tile([P, TPIECE], F32, tag="r4")
                            nc.vector.tensor_tensor(t3[:], psB[:], cos, ALU.mult)
                            nc.vector.tensor_tensor(t4[:], psA[:], sin, ALU.mult)
                            nc.vector.tensor_tensor(
                                dst[:, hc0 + 1, t0:t0 + TPIECE], t3[:], t4[:],
                                ALU.add)

                        # Q projections: 4 h-chunks (2 heads x 2 halves)
                        for hh in range(2):
                            ps = []
                            for half in range(2):
                                hc = hh * 2 + half
                                pq = ppsum.tile([P, TPIECE], F32, tag="pq")
                                for dc in range(DC):
                                    nc.tensor.matmul(
                                        pq[:],
                                        wq_sb[:, dc, hc * P:(hc + 1) * P],
                                        xt[:, dc, :],
                                        start=(dc == 0), stop=(dc == DC - 1))
                                ps.append(pq)
                            rope(qt, hh * 2, ps[0], ps[1], rq_sb)
                        # K projection: 2 h-chunks
                        ps = []
                        for half in range(2):
                            pk = ppsum.tile([P, TPIECE], F32, tag="pq")
                            for dc in range(DC):
                                nc.tensor.matmul(
                                    pk[:],
                                    wk_sb[:, dc, half * P:(half + 1) * P],
                                    xt[:, dc, :],
                                    start=(dc == 0), stop=(dc == DC - 1))
                            ps.append(pk)
                        rope(kt, 0, ps[0], ps[1], rk_sb)
                        # V projection: natural layout [t, h]
                        for tc4 in range(TPIECE // P):
                            pv = vpsum.tile([P, H], F32, tag="pv")
                            for dc in range(DC):
                                nc.tensor.matmul(
                                    pv[:],
                                    xt[:, dc, tc4 * P:(tc4 + 1) * P],
                                    wv_sb[:, dc, :],
                                    start=(dc == 0), stop=(dc == DC - 1))
                            nc.vector.tensor_copy(
                                out=vsb[:, pi * (TPIECE // P) + tc4, :],
                                in_=pv[:])

            # ============ banded attention in 2 rounds + A2A ==============
            with (
                tc.tile_pool(name="attn_sb", bufs=2) as apool,
                tc.tile_pool(name="stats", bufs=3) as spool,
                tc.tile_pool(name="attn_ps", bufs=2, space="PSUM") as apsum,
                tc.tile_pool(name="enc_ps", bufs=2, space="PSUM") as encpsum,
                tc.tile_pool(name="oproj", bufs=2) as opool,
                tc.tile_pool(name="enc_full", bufs=1) as efpool,
                tc.tile_pool(name="oproj_ps", bufs=2, space="PSUM") as opsum,
            ):
                def attend(b, hh, i):
                    qt, kt, vsb = qts[b], kts[b], vsbs[b]
                    lo, hi = _band(i, mode)
                    segs = _segments(lo, hi)
                    tstrips, rsums = [], []
                    for si, (j0, j1) in enumerate(segs):
                        nj = j1 - j0 + 1
                        w = nj * P
                        qk = apsum.tile([P, SEGMAX * P], F32, tag="qk")
                        # QK^T in 512-wide blocks, contiguous group per block
                        for blk0 in range(0, w, 512):
                            bw = min(512, w - blk0)
                            for c in range(2):
                                nc.tensor.matmul(
                                    qk[:, blk0:blk0 + bw],
                                    qt[:, hh * 2 + c, i * P:(i + 1) * P],
                                    kt[:, c,
                                       j0 * P + blk0:j0 * P + blk0 + bw],
                                    start=(c == 0), stop=(c == 1))
                        ts_ = apool.tile([P, w], F32, tag=f"tstrip{si}")
                        nc.scalar.activation(ts_[:, :w], qk[:, :w], AF.Tanh,
                                             scale=1.0 / SOFT_CAP)
                        for jj in range(nj):
                            j = j0 + jj
                            m = None
                            if j == i - WTILES:
                                m = upperstrict
                            elif j == i and mode == "tril":
                                m = causal
                            elif j == i + WTILES and mode == "ones":
                                m = lowerstrict
                            if m is not None:
                                sl = slice(jj * P, (jj + 1) * P)
                                nc.vector.tensor_tensor(
                                    ts_[:, sl], ts_[:, sl], m[:], ALU.add)
                        rs = spool.tile([P, 1], F32, tag=f"rs{si}")
                        nc.scalar.activation(ts_[:, :w], ts_[:, :w], AF.Exp,
                                             scale=SOFT_CAP, accum_out=rs[:])
                        tstrips.append(ts_)
                        rsums.append(rs)
                    rtot = rsums[0]
                    for si in range(1, len(rsums)):
                        nrt = spool.tile([P, 1], F32, tag=f"rtot{si}")
                        nc.vector.tensor_tensor(nrt[:], rtot[:],
                                                rsums[si][:], ALU.add)
                        rtot = nrt
                    rinv = spool.tile([P, 1], F32, tag="rinv")
                    nc.vector.reciprocal(rinv[:], rtot[:])
                    # normalize -> transpose (f32 PE path)
                    pts = []
                    for si, (j0, j1) in enumerate(segs):
                        nj = j1 - j0 + 1
                        w = nj * P
                        pn = apool.tile([P, w], F32, tag=f"pn{si}")
                        nc.vector.tensor_scalar_mul(
                            pn[:, :w], tstrips[si][:, :w], rinv[:])
                        tr = apsum.tile([P, SEGMAX * P], F32, tag="qk")
                        for jj in range(nj):
                            nc.tensor.transpose(tr[:, jj * P:(jj + 1) * P],
                                                pn[:, jj * P:(jj + 1) * P],
                                                ident[:])
                        pt = apool.tile([P, w], BF16, tag=f"pt{si}")
                        nc.vector.tensor_copy(out=pt[:, :w], in_=tr[:, :w])
                        pts.append(pt)
                    # PV: natural enc [q, 256], one contiguous group
                    njs = [j1 - j0 + 1 for j0, j1 in segs]
                    ntot = sum(njs)
                    encp = encpsum.tile([P, H], F32, tag="enc")
                    n = 0
                    for si, (j0, j1) in enumerate(segs):
                        for jj in range(njs[si]):
                            j = j0 + jj
                            nc.tensor.matmul(
                                encp[:], pts[si][:, jj * P:(jj + 1) * P],
                                vsb[:, j, :],
                                start=(n == 0), stop=(n == ntot - 1))
                            n += 1
                    encsb = apool.tile([P, H], BF16, tag="encsb")
                    nc.vector.tensor_copy(out=encsb[:], in_=encp[:])
                    gtok = b * T + i * P
                    jb = gtok // TPC
                    ch = (gtok % TPC) // CHTOK
                    toff = gtok % CHTOK
                    nc.sync.dma_start(
                        a2a_in[ch][jb, toff:toff + P,
                                   hh * H:(hh + 1) * H], encsb[:])

                for ch in range(2):
                    for b in range(B):
                        for hh in range(2):
                            for i in range(TQ):
                                if (i % 4) // 2 == ch:
                                    attend(b, hh, i)
                    nc.gpsimd.collective_compute(
                        "AllToAll", ALU.bypass,
                        replica_groups=[list(range(NCORES))],
                        ins=[a2a_in[ch][:].opt()],
                        outs=[a2a_out[ch][:].opt()])

                # single-pass output projection; token halves come from the
                # two A2A chunks so the first half can start before chunk 1
                # has landed
                efs = []
                for ch in range(2):
                    ef = efpool.tile([P, NHC, CHTOK], BF16, tag=f"ef{ch}")
                    for j in range(NCORES):
                        for hc in range(4):
                            nc.sync.dma_start_transpose(
                                ef[:, j * 4 + hc, :],
                                a2a_out[ch][j, :, hc * P:(hc + 1) * P])
                    efs.append(ef)
                for dp in range(D // DP):
                    wo_sb = opool.tile([P, NHC, DP], BF16, tag="wo")
                    nc.sync.dma_start(
                        wo_sb[:],
                        wo[:, :, dp * DP:(dp + 1) * DP].rearrange(
                            "c p d -> p c d"))
                    for tc4 in range(TPC // P):
                        ef = efs[tc4 // 2]
                        tc2 = tc4 % 2
                        po = opsum.tile([P, DP], F32, tag="po")
                        for hc in range(NHC):
                            nc.tensor.matmul(
                                po[:], ef[:, hc, tc2 * P:(tc2 + 1) * P],
                                wo_sb[:, hc, :],
                                start=(hc == 0), stop=(hc == NHC - 1))
                        osb = opool.tile([P, DP], F32, tag="osb")
                        nc.vector.tensor_copy(out=osb[:], in_=po[:])
                        nc.sync.dma_start(
                            out[tc4 * P:(tc4 + 1) * P, dp * DP:(dp + 1) * DP],
                            osb[:])

    nc.compile()
    return nc


def _rope_tables(pos, scale):
    """pos: [T] int array -> [2, 128, T] bf16 (cos;sin), scaled."""
    frac = 2.0 * np.arange(H // 2, dtype=np.float64) / H
    timescale = ROPE_BASE ** frac                      # [128]
    sinusoid = pos.astype(np.float64)[None, :] / timescale[:, None]  # [128,T]
    tabs = np.stack([np.cos(sinusoid), np.sin(sinusoid)]) * scale
    return tabs.astype(ml_dtypes.bfloat16)


def _reference_host(x, segment_pos, attn_mask, w_q, w_kv, w_o):
    """Slow but fully general fallback (numpy)."""
    xb = x.astype(np.float32)
    q = np.einsum('btd,ndh->btnh', xb, w_q)
    k = np.einsum('bsd,kdh->bskh', xb, w_kv[0])
    v = np.einsum('bsd,kdh->bskh', xb, w_kv[1])

    def rope(t, positions):
        hd = t.shape[-1]
        frac = 2.0 * np.arange(hd // 2, dtype=np.float32) / hd
        ts_ = ROPE_BASE ** frac
        sinusoid = positions.astype(np.float32)[..., None] / ts_
        sinusoid = sinusoid[..., None, :]
        s, c = np.sin(sinusoid), np.cos(sinusoid)
        first, second = np.split(t, 2, axis=-1)
        return np.concatenate([first * c - second * s,
                               second * c + first * s], axis=-1)

    q = rope(q, segment_pos) * SCALAR
    k = rope(k, segment_pos)
    qg = q.reshape(B, T, NKV, 2, H)
    logits = np.einsum('btkgh,bskh->btkgs', qg, k).reshape(B, T, NQ, T)
    logits = np.tanh(logits / SOFT_CAP) * SOFT_CAP
    pos_s = np.arange(T)[None, None, :]
    pos_t = segment_pos[:, :, None]
    sliding = (pos_s > pos_t - WINDOW) & (pos_s < pos_t + WINDOW)
    mask = np.logical_and(attn_mask, sliding)
    padded = np.where(mask[:, :, None, :], logits, -np.inf)
    padded -= padded.max(axis=-1, keepdims=True)
    e = np.exp(padded)
    probs = (e / e.sum(axis=-1, keepdims=True)).astype(np.float32)
    v_exp = np.repeat(v, NQ // NKV, axis=2)            # [B,T,NQ,H]
    enc = np.einsum('btns,bsnh->btnh', probs, v_exp)
    return np.einsum('btnh,nhd->btd', enc, w_o).astype(np.float32)


_GRAPH_CACHE = {}


def kernel(x, segment_pos, attn_mask, w_q, w_kv, w_o):
    global last_result
    x = np.asarray(x)
    segment_pos = np.asarray(segment_pos)
    attn_mask = np.asarray(attn_mask)
    w_q = np.asarray(w_q, dtype=np.float32)
    w_kv = np.asarray(w_kv, dtype=np.float32)
    w_o = np.asarray(w_o, dtype=np.float32)

    arange = np.broadcast_to(np.arange(T, dtype=segment_pos.dtype), (B, T))
    std_pos = np.array_equal(segment_pos, arange)
    tril = np.broadcast_to(np.tril(np.ones((T, T), dtype=bool)), (B, T, T))
    if attn_mask.all():
        mode = "ones"
    elif np.array_equal(attn_mask, tril):
        mode = "tril"
    else:
        mode = None
    if not std_pos or mode is None:
        return _reference_host(x, segment_pos, attn_mask, w_q, w_kv, w_o)

    if mode not in _GRAPH_CACHE:
        _GRAPH_CACHE[mode] = build(mode)
    nc = _GRAPH_CACHE[mode]

    bf = ml_dtypes.bfloat16
    xT = np.ascontiguousarray(x.reshape(TOK, D).T).astype(bf)    # [D, TOK]
    pos = segment_pos[0]
    ropeq = np.ascontiguousarray(_rope_tables(pos, SCALAR))
    ropek = np.ascontiguousarray(_rope_tables(pos, 1.0))
    wo_all = np.ascontiguousarray(
        w_o.reshape(NHC, P, D)).astype(bf)

    in_maps = []
    for c in range(NCORES):
        wq_c = np.ascontiguousarray(
            np.concatenate([w_q[2 * c], w_q[2 * c + 1]], axis=1)).astype(bf)
        wk_c = np.ascontiguousarray(w_kv[0, c]).astype(bf)
        wv_c = np.ascontiguousarray(w_kv[1, c]).astype(bf)
        in_maps.append({
            "xT": xT, "wq": wq_c, "wk": wk_c, "wv": wv_c, "wo": wo_all,
            "ropeq": ropeq, "ropek": ropek,
        })

    trace = os.environ.get("KTRACE", "0") == "1"
    res = run_bass_kernel_spmd(nc, in_maps, core_ids=list(range(NCORES)),
                               trace=trace)
    last_result = res
    outs = [res.results[c]["out"] for c in range(NCORES)]
    return np.concatenate(outs, axis=0).reshape(B, T, D).astype(np.float32)
